# revision 1
# baseline (speedup 1.0000x reference)
"""Trainium2 Bass kernel for nn_COCQCNN_layer (quantum 2x2-patch circuit layer).

Full inputs: x [16, 3, 256, 256] f32, thetas [12] f32, phis [3] f32.
Output: [16, 1, 128, 128] f32 = <Z_0> per 2x2 patch of a 5-qubit circuit.

Algorithm (X-basis form): for each patch, the 4 per-patch RX gates of a layer
are jointly diagonal in the Hadamard basis of wires 1-4: amplitude (bcde)
picks up phase e^{-i sigma(bcde)}, sigma = sum_w +-theta_w/2. The fixed
two-qubit gates (thetas/phis-dependent only) are folded into per-layer 64x64
real matrices applied on the TensorEngine. Per 1024-patch tile:
  sigma matmul (PE, fp32r exact via hi/lo angle split) -> range wrap to
  [-pi,pi] (DVE) -> Sin (ACT) -> cos/sin broadcast matmuls (PE) -> two
  elementwise state multiplies (DVE) -> fixed-layer matmuls w/ PSUM
  accumulation (PE) -> expectation via product + reduction matmul.

Sharding: pure data parallel over patches; 8 cores x 32 tiles x 1024 patches.
"""
import sys
import os

sys.path.insert(0, '/opt/trn_rl_repo')

import numpy as np

KAPPA = 2.0 ** -2.5
PI = np.pi
N_CORES = 8
TILES_PER_CORE = 32
P_TOTAL = 262144          # 16 * 128 * 128
_REPEAT = int(os.environ.get("KERNEL_REPEAT", "1"))
_CACHE = {}


# ----------------------------------------------------------------------------
# host-side constant construction
# ----------------------------------------------------------------------------

def _split_hilo(x, bits=10):
    """hi keeps top `bits` stored mantissa bits (truncate); lo = x - hi.
    Both parts are exactly representable in the PE's fp32r (TF32-like)."""
    x = np.asarray(x, np.float32)
    u = x.view(np.uint32)
    mask = np.uint32((0xFFFFFFFF << (23 - bits)) & 0xFFFFFFFF)
    hi = (u & mask).view(np.float32)
    lo = (x.astype(np.float64) - hi.astype(np.float64)).astype(np.float32)
    return hi, lo


def _kron_list(mats):
    out = np.array([[1.0]], np.complex128)
    for m in mats:
        out = np.kron(out, m)
    return out


def _embed(gate2q, wires):
    U = np.zeros((32, 32), np.complex128)
    wc, wt = wires
    for idx_in in range(32):
        bits_in = [(idx_in >> (4 - w)) & 1 for w in range(5)]
        for co in range(2):
            for to in range(2):
                amp = gate2q[co, to, bits_in[wc], bits_in[wt]]
                if amp == 0:
                    continue
                bits_out = list(bits_in)
                bits_out[wc] = co
                bits_out[wt] = to
                idx_out = sum(bits_out[w] << (4 - w) for w in range(5))
                U[idx_out, idx_in] += amp
    return U


def _x_theta(theta):
    e = np.exp(0.5j * theta)
    return np.array([[0, -1j * e], [-1j * np.conj(e), 0]], np.complex128)


def _cu(theta):
    cu = np.zeros((2, 2, 2, 2), np.complex128)
    cu[0, :, 0, :] = np.eye(2)
    cu[1, :, 1, :] = _x_theta(theta)
    return cu


def _cphase(phi):
    g = np.zeros((2, 2, 2, 2), np.complex128)
    g[0, :, 0, :] = np.eye(2)
    g[1, 0, 1, 0] = 1.0
    g[1, 1, 1, 1] = np.exp(1j * phi)
    return g


def _fixed_layer_matrices(thetas, phis):
    H = np.array([[1, 1], [1, -1]], np.complex128) / np.sqrt(2)
    G = _kron_list([np.eye(2), H, H, H, H])
    pairs = [(1, 2), (2, 3), (3, 4), (4, 1)]
    mats = []
    for l in range(3):
        F = np.eye(32, dtype=np.complex128)
        for w in range(4):
            F = _embed(_cu(thetas[4 * l + w]), pairs[w]) @ F
        F = _embed(_cphase(phis[l]), (0, 1)) @ F
        mats.append(G @ F @ G)
    return mats


def _realify(M):
    n = M.shape[0]
    R = np.zeros((2 * n, 2 * n))
    R[0::2, 0::2] = M.real
    R[0::2, 1::2] = -M.imag
    R[1::2, 0::2] = M.imag
    R[1::2, 1::2] = M.real
    return R


def _expand_group(M64):
    """64x64 real on (a,b,r) -> 128x128 on device rows a*64+g*32+b*2+r."""
    F = np.zeros((128, 128))
    ar = np.arange(2)
    comp = ((ar[:, None, None] * 16 + np.arange(16)[None, :, None]) * 2
            + np.arange(2)[None, None, :])          # [a, b, r] -> comp idx
    row = (ar[:, None, None] * 64 + np.arange(16)[None, :, None] * 2
           + np.arange(2)[None, None, :])           # within group g=0
    comp = comp.reshape(-1)
    row = row.reshape(-1)
    for g in range(2):
        F[np.ix_(row + g * 32, row + g * 32)] = M64[np.ix_(comp, comp)]
    return F


def _build_constants(thetas, phis):
    thetas = np.asarray(thetas, np.float64)
    phis = np.asarray(phis, np.float64)
    Ft = _fixed_layer_matrices(thetas, phis)
    Fhat = [_expand_group(_realify(M)) for M in Ft]

    SWAP = np.zeros((128, 128))
    for a in range(2):
        for g in range(2):
            for b in range(16):
                for r in range(2):
                    SWAP[a * 64 + g * 32 + b * 2 + r,
                         a * 64 + g * 32 + b * 2 + (1 - r)] = 1.0

    def s_w(b, w):
        return 1.0 if ((b >> (3 - w)) & 1) == 0 else -1.0

    pi2_hi, pi2_lo = _split_hilo(np.float32(PI / 2))
    c_sigma = np.zeros((18, 64))
    for g in range(2):
        for t in range(2):
            for b in range(16):
                prow = g * 32 + t * 16 + b
                for w in range(4):
                    c_sigma[g * 4 + w, prow] = 0.5 * s_w(b, w)
                    c_sigma[8 + g * 4 + w, prow] = 0.5 * s_w(b, w)
                if t == 0:
                    c_sigma[16, prow] = float(pi2_hi)
                    c_sigma[17, prow] = float(pi2_lo)
    c_sig2 = np.zeros((36, 128))
    c_sig2[0:18, 0:64] = c_sigma
    c_sig2[18:36, 64:128] = c_sigma

    B0 = np.zeros((128, 64))
    Mc = np.zeros((128, 64))
    Ms = np.zeros((128, 64))
    for a in range(2):
        for g in range(2):
            for b in range(16):
                B0[a * 64 + g * 32 + b * 2 + 0, g * 32 + b] = KAPPA
                B0[a * 64 + g * 32 + b * 2 + 1, g * 32 + 16 + b] = -KAPPA
                Mc[a * 64 + g * 32 + b * 2 + 0, g * 32 + b] = 1.0
                Mc[a * 64 + g * 32 + b * 2 + 1, g * 32 + b] = 1.0
                Ms[a * 64 + g * 32 + b * 2 + 0, g * 32 + 16 + b] = -1.0
                Ms[a * 64 + g * 32 + b * 2 + 1, g * 32 + 16 + b] = 1.0
    build = Fhat[0] @ B0                     # [128 state, 64 P]

    def embed_tt(M, tt):
        """[128 state, 64 P] -> lhsT [128 K(P2-rows), 128 M] for tile tt."""
        L = np.zeros((128, 128), np.float32)
        L[64 * tt:64 * tt + 64, :] = M.T
        return L

    def hilo(M):
        return _split_hilo(M)

    b_h, b_l = hilo(build)
    c_bld2 = np.stack([embed_tt(b_h, 0), embed_tt(b_l, 0),
                       embed_tt(b_h, 1), embed_tt(b_l, 1)])
    c_bc2 = np.stack([embed_tt(Mc.astype(np.float32), 0),
                      embed_tt(Mc.astype(np.float32), 1)])
    c_bs2 = np.stack([embed_tt(Ms.astype(np.float32), 0),
                      embed_tt(Ms.astype(np.float32), 1)])
    f_list = []
    for M in (Fhat[1], Fhat[1] @ SWAP, Fhat[2], Fhat[2] @ SWAP):
        h, l = hilo(M)
        f_list += [h.T, l.T]
    c_f = np.stack(f_list)                   # [8, 128, 128]

    c_ev4 = np.zeros((4, 64, 8), np.float32)
    for sl in range(4):
        for g in range(2):
            c_ev4[sl, g * 32:(g + 1) * 32, 2 * sl + g] = 2.0

    return dict(
        c_sig=np.ascontiguousarray(c_sig2.astype(np.float32)),
        c_bld=np.ascontiguousarray(c_bld2),
        c_bc=np.ascontiguousarray(c_bc2),
        c_bs=np.ascontiguousarray(c_bs2),
        c_f=np.ascontiguousarray(c_f.astype(np.float32)),
        c_ev=np.ascontiguousarray(c_ev4),
    )


def _angle_blocks(pix):
    """pix [P, 12] f32 -> A [P/2048 macros, 3, 36, 512] f32.
    Per (macro, layer): rows [tileA: hi(g*4+w) x8, lo x8, 1, 1][tileB: same] (36)."""
    n_macro = pix.shape[0] // 2048
    hi, lo = _split_hilo(pix)
    # [macro, tt, g, n, 12] -> [macro, 12, tt, g, n]
    hi = hi.reshape(n_macro, 2, 2, 512, 12).transpose(0, 4, 1, 2, 3)
    lo = lo.reshape(n_macro, 2, 2, 512, 12).transpose(0, 4, 1, 2, 3)
    A = np.zeros((n_macro, 3, 36, 512), np.float32)
    for l in range(3):
        for tt in range(2):
            for g in range(2):
                for w in range(4):
                    A[:, l, tt * 18 + g * 4 + w, :] = hi[:, 4 * l + w, tt, g, :]
                    A[:, l, tt * 18 + 8 + g * 4 + w, :] = lo[:, 4 * l + w, tt, g, :]
            A[:, l, tt * 18 + 16, :] = 1.0
            A[:, l, tt * 18 + 17, :] = 1.0
    return A


# ----------------------------------------------------------------------------
# device program
# ----------------------------------------------------------------------------

def _build_nc(n_tiles=TILES_PER_CORE, repeat=1):
    """n_tiles = old 1024-patch tiles per core; must be divisible by 4."""
    import contextlib
    import concourse.mybir as mybir
    from concourse import bacc
    from concourse.tile import TileContext

    F32 = mybir.dt.float32
    F32R = mybir.dt.float32r
    AF = mybir.ActivationFunctionType

    assert n_tiles % 4 == 0
    n_macro = n_tiles // 2

    nc = bacc.Bacc(None, target_bir_lowering=False, debug=False)
    ang_d = nc.declare_dram_parameter("ang", [n_macro, 3, 36, 512], F32R,
                                      isOutput=False)
    csig_d = nc.declare_dram_parameter("c_sig", [36, 128], F32R, isOutput=False)
    cbld_d = nc.declare_dram_parameter("c_bld", [4, 128, 128], F32R, isOutput=False)
    cbc_d = nc.declare_dram_parameter("c_bc", [2, 128, 128], F32R, isOutput=False)
    cbs_d = nc.declare_dram_parameter("c_bs", [2, 128, 128], F32R, isOutput=False)
    cf_d = nc.declare_dram_parameter("c_f", [8, 128, 128], F32R, isOutput=False)
    cev_d = nc.declare_dram_parameter("c_ev", [4, 64, 8], F32R, isOutput=False)
    ev_d = nc.declare_dram_parameter("ev", [n_tiles // 4, 8, 512], F32,
                                     isOutput=True)

    BA = int(os.environ.get("BUFS_ANG", "6"))
    BW = int(os.environ.get("BUFS_WRK", "6"))
    BP = int(os.environ.get("BUFS_PSIS", "6"))
    BM = int(os.environ.get("BUFS_MMT", "6"))
    PS_SIG = int(os.environ.get("PS_SIG", "2"))
    PS_BCBS = int(os.environ.get("PS_BCBS", "2"))
    PS_PSI = int(os.environ.get("PS_PSI", "3"))
    PSI_PSUM = os.environ.get("PSI_PSUM", "0") == "1"

    with TileContext(nc) as tc:
        with (
            tc.tile_pool(name="const", bufs=1) as cpool,
            tc.tile_pool(name="angp", bufs=BA) as angp,
            tc.tile_pool(name="wrk", bufs=BW) as wrk,
            tc.tile_pool(name="psis", bufs=BP) as psis,
            tc.tile_pool(name="mmt", bufs=BM) as mmt,
            tc.tile_pool(name="evs", bufs=2) as evs,
            tc.tile_pool(name="sigp", bufs=PS_SIG, space="PSUM") as sigp,
            tc.tile_pool(name="bcbs", bufs=PS_BCBS, space="PSUM") as bcbs,
            tc.tile_pool(name="psip", bufs=PS_PSI, space="PSUM") as psip,
            tc.tile_pool(name="evp", bufs=1, space="PSUM") as evp,
        ):
            c_sig = cpool.tile([36, 128], F32R)
            nc.sync.dma_start(out=c_sig[:], in_=csig_d[:])
            c_bld = []
            for k in range(4):
                tb = cpool.tile([128, 128], F32R, tag=f"bld{k}")
                nc.sync.dma_start(out=tb[:], in_=cbld_d[k])
                c_bld.append(tb)
            c_bc = []
            c_bs = []
            for tt in range(2):
                t1 = cpool.tile([128, 128], F32R, tag=f"bc{tt}")
                nc.sync.dma_start(out=t1[:], in_=cbc_d[tt])
                c_bc.append(t1)
                t2 = cpool.tile([128, 128], F32R, tag=f"bs{tt}")
                nc.sync.dma_start(out=t2[:], in_=cbs_d[tt])
                c_bs.append(t2)
            c_f = []
            for k in range(8):
                tf = cpool.tile([128, 128], F32R, tag=f"f{k}")
                nc.sync.dma_start(out=tf[:], in_=cf_d[k])
                c_f.append(tf)
            c_ev = []
            for sl in range(4):
                te = cpool.tile([64, 8], F32R, tag=f"ev{sl}")
                nc.sync.dma_start(out=te[:], in_=cev_d[sl])
                c_ev.append(te)

            rep_ctx = (tc.For_i(0, repeat, 1) if repeat > 1
                       else contextlib.nullcontext())
            with rep_ctx:
                evt = None
                for m in range(n_macro):
                    a_ls = []
                    for l in range(3):
                        a_l = angp.tile([36, 512], F32R, tag=f"ang{l}")
                        nc.gpsimd.dma_start(out=a_l[:], in_=ang_d[m, l])
                        a_ls.append(a_l)

                    psi_s = [None, None]
                    psi_ab = [None, None]
                    for l in range(3):
                        sig = sigp.tile([128, 512], F32, tag="sig")
                        nc.tensor.matmul(sig[:], c_sig[:], a_ls[l][:],
                                         start=True, stop=True)
                        w = wrk.tile([128, 512], F32, tag="w")
                        nc.vector.add_range_wrap(
                            w[:], sig[:], shift=0.0, bound=PI, period=2 * PI)
                        p = wrk.tile([128, 512], F32R, tag="p")
                        nc.scalar.activation(p[:], w[:], AF.Sin)

                        for tt in range(2):
                            psi_p = psip.tile([128, 512], F32, tag="psi")
                            if l == 0:
                                nc.tensor.matmul(psi_p[:], c_bld[2 * tt][:], p[:],
                                                 start=True, stop=False)
                                nc.tensor.matmul(psi_p[:], c_bld[2 * tt + 1][:],
                                                 p[:], start=False, stop=True)
                            else:
                                bc = bcbs.tile([128, 512], F32, tag="bcbs")
                                nc.tensor.matmul(bc[:], c_bc[tt][:], p[:],
                                                 start=True, stop=True)
                                bs = bcbs.tile([128, 512], F32, tag="bcbs")
                                nc.tensor.matmul(bs[:], c_bs[tt][:], p[:],
                                                 start=True, stop=True)
                                if PSI_PSUM:
                                    bcs = psis.tile([128, 512], F32, tag="bcs")
                                    nc.scalar.copy(out=bcs[:], in_=bc[:])
                                    bss = psis.tile([128, 512], F32, tag="bss")
                                    nc.scalar.copy(out=bss[:], in_=bs[:])
                                    m1 = mmt.tile([128, 512], F32R, tag="m")
                                    nc.vector.tensor_mul(m1[:], bcs[:],
                                                         psi_s[tt][:])
                                    m2 = mmt.tile([128, 512], F32R, tag="m")
                                    nc.vector.tensor_mul(m2[:], bss[:],
                                                         psi_s[tt][:])
                                else:
                                    m1 = mmt.tile([128, 512], F32R, tag="m")
                                    nc.vector.tensor_mul(m1[:], bc[:],
                                                         psi_s[tt][:])
                                    m2 = mmt.tile([128, 512], F32R, tag="m")
                                    nc.vector.tensor_mul(m2[:], bs[:],
                                                         psi_s[tt][:])
                                base = 4 * (l - 1)
                                nc.tensor.matmul(psi_p[:], c_f[base + 0][:],
                                                 m1[:], start=True, stop=False)
                                nc.tensor.matmul(psi_p[:], c_f[base + 1][:],
                                                 m1[:], start=False, stop=False)
                                nc.tensor.matmul(psi_p[:], c_f[base + 2][:],
                                                 m2[:], start=False, stop=False)
                                nc.tensor.matmul(psi_p[:], c_f[base + 3][:],
                                                 m2[:], start=False, stop=True)
                            if l < 2:
                                if PSI_PSUM:
                                    psi_s[tt] = psi_p
                                else:
                                    ps_t = psis.tile([128, 512], F32,
                                                     tag="psis")
                                    nc.scalar.copy(out=ps_t[:], in_=psi_p[:])
                                    psi_s[tt] = ps_t
                            elif os.environ.get("Q_PSUM2", "0") == "1":
                                psi_ab[tt] = (psi_p, psi_p)
                            else:
                                pa = psis.tile([64, 512], F32, tag="psia")
                                nc.scalar.copy(out=pa[:], in_=psi_p[0:64, :])
                                psi_ab[tt] = (pa, psi_p)

                    for tt in range(2):
                        sl = (2 * m + tt) % 4
                        if sl == 0:
                            evt = evp.tile([8, 512], F32, tag="ev")
                        q = mmt.tile([64, 512], F32R, tag="q")
                        if os.environ.get("Q_PSUM2", "0") == "1":
                            nc.vector.tensor_mul(q[:], psi_ab[tt][0][0:64, :],
                                                 psi_ab[tt][1][64:128, :])
                        else:
                            nc.vector.tensor_mul(q[:], psi_ab[tt][0][:],
                                                 psi_ab[tt][1][64:128, :])
                        nc.tensor.matmul(evt[:], c_ev[sl][:], q[:],
                                         start=(sl == 0), stop=(sl == 3))
                        if sl == 3:
                            g4 = (2 * m + tt) // 4
                            ev_s = evs.tile([8, 512], F32, tag="evs")
                            nc.scalar.copy(out=ev_s[:], in_=evt[:])
                            nc.sync.dma_start(out=ev_d[g4], in_=ev_s[:])

    nc.finalize()
    return nc


def _get_nc(repeat=_REPEAT):
    key = ("nc", repeat)
    if key not in _CACHE:
        _CACHE[key] = _build_nc(repeat=repeat)
    return _CACHE[key]


# ----------------------------------------------------------------------------
# entry point
# ----------------------------------------------------------------------------

def kernel(x, thetas, phis):
    from concourse.bass_utils import run_bass_kernel_spmd

    x = np.asarray(x, np.float32)
    thetas = np.asarray(thetas, np.float32)
    phis = np.asarray(phis, np.float32)
    B, C, H, W = x.shape
    H2, W2 = H // 2, W // 2
    pix = (x.reshape(B, 3, H2, 2, W2, 2)
             .transpose(0, 2, 4, 1, 3, 5)
             .reshape(B * H2 * W2, 12))

    A = _angle_blocks(pix)                    # [128 macros, 3, 36, 512]
    consts = _build_constants(thetas, phis)
    per_core = A.shape[0] // N_CORES
    in_maps = [{"ang": np.ascontiguousarray(A[c * per_core:(c + 1) * per_core]),
                **consts} for c in range(N_CORES)]

    nc = _get_nc()
    res = run_bass_kernel_spmd(nc, in_maps, list(range(N_CORES)))
    # ev_d [n_tiles//4, 8, 512]: row 2*slot+g of group g4 -> old tile 4*g4+slot
    evs = [res.results[c]["ev"].reshape(-1, 4, 2, 512).reshape(-1)
           for c in range(N_CORES)]
    ev = np.concatenate(evs)
    return ev.reshape(B, 1, H2, W2).astype(np.float32)



# revision 12
# speedup vs baseline: 3.2953x; 3.2953x over previous
"""Trainium2 Bass kernel for nn_COCQCNN_layer (quantum 2x2-patch circuit layer).

Full inputs: x [16, 3, 256, 256] f32, thetas [12] f32, phis [3] f32.
Output: [16, 1, 128, 128] f32 = <Z_0> per 2x2 patch of a 5-qubit circuit.

Algorithm (X-basis form): the 4 per-patch RX gates of a layer are jointly
diagonal in the Hadamard basis of wires 1-4: amplitude (b in {0,1}^4) picks up
phase e^{-i sigma_b}, sigma_b = sum_w +-a_w/2. Complement symmetry
sigma_{~b} = -sigma_b means only 8 of 16 (cos, sin) rows are independent, so
one [128, 512] sigma tile (8 patch-groups x {cos,sin} x 8 b-reps) serves 4096
patches (a "pair" of 2048-patch macros). Fixed per-layer 64x64 real matrices
(thetas/phis only) apply on the TensorEngine as packed 128x128 fp16 matmuls.
Per-patch data enters via cos/sin broadcast matmuls + fp16 elementwise
multiplies (DVE 2x mode; PSUM->SBUF fp16 staging copies split over ACT/DVE).

Sharding: pure data parallel over patches; 8 cores x 8 pairs x 4096 patches.
"""
import sys
import os

sys.path.insert(0, '/opt/trn_rl_repo')

import numpy as np

KAPPA = 2.0 ** -2.5
PI = np.pi
N_CORES = 8
TILES_PER_CORE = 32           # old 1024-patch tiles; 4 per pair
P_TOTAL = 262144              # 16 * 128 * 128
_REPEAT = int(os.environ.get("KERNEL_REPEAT", "1"))
_CACHE = {}


# ----------------------------------------------------------------------------
# host-side constant construction
# ----------------------------------------------------------------------------

def _kron_list(mats):
    out = np.array([[1.0]], np.complex128)
    for m in mats:
        out = np.kron(out, m)
    return out


def _embed(gate2q, wires):
    U = np.zeros((32, 32), np.complex128)
    wc, wt = wires
    for idx_in in range(32):
        bits_in = [(idx_in >> (4 - w)) & 1 for w in range(5)]
        for co in range(2):
            for to in range(2):
                amp = gate2q[co, to, bits_in[wc], bits_in[wt]]
                if amp == 0:
                    continue
                bits_out = list(bits_in)
                bits_out[wc] = co
                bits_out[wt] = to
                idx_out = sum(bits_out[w] << (4 - w) for w in range(5))
                U[idx_out, idx_in] += amp
    return U


def _x_theta(theta):
    e = np.exp(0.5j * theta)
    return np.array([[0, -1j * e], [-1j * np.conj(e), 0]], np.complex128)


def _cu(theta):
    cu = np.zeros((2, 2, 2, 2), np.complex128)
    cu[0, :, 0, :] = np.eye(2)
    cu[1, :, 1, :] = _x_theta(theta)
    return cu


def _cphase(phi):
    g = np.zeros((2, 2, 2, 2), np.complex128)
    g[0, :, 0, :] = np.eye(2)
    g[1, 0, 1, 0] = 1.0
    g[1, 1, 1, 1] = np.exp(1j * phi)
    return g


def _fixed_layer_matrices(thetas, phis):
    H = np.array([[1, 1], [1, -1]], np.complex128) / np.sqrt(2)
    G = _kron_list([np.eye(2), H, H, H, H])
    pairs = [(1, 2), (2, 3), (3, 4), (4, 1)]
    mats = []
    for l in range(3):
        F = np.eye(32, dtype=np.complex128)
        for w in range(4):
            F = _embed(_cu(thetas[4 * l + w]), pairs[w]) @ F
        F = _embed(_cphase(phis[l]), (0, 1)) @ F
        mats.append(G @ F @ G)
    return mats


def _realify(M):
    n = M.shape[0]
    R = np.zeros((2 * n, 2 * n))
    R[0::2, 0::2] = M.real
    R[0::2, 1::2] = -M.imag
    R[1::2, 0::2] = M.imag
    R[1::2, 1::2] = M.real
    return R


def _expand_group(M64):
    """64x64 real on (a,b,r) -> 128x128 on device rows a*64+g*32+b*2+r."""
    F = np.zeros((128, 128))
    ar = np.arange(2)
    comp = ((ar[:, None, None] * 16 + np.arange(16)[None, :, None]) * 2
            + np.arange(2)[None, None, :])
    row = (ar[:, None, None] * 64 + np.arange(16)[None, :, None] * 2
           + np.arange(2)[None, None, :])
    comp = comp.reshape(-1)
    row = row.reshape(-1)
    for g in range(2):
        F[np.ix_(row + g * 32, row + g * 32)] = M64[np.ix_(comp, comp)]
    return F


def _s_w(b, w):
    return 1.0 if ((b >> (3 - w)) & 1) == 0 else -1.0


def _j_of_b(b):
    return b if b < 8 else 15 - b


def _sgn_of_b(b):
    return 1.0 if b < 8 else -1.0


def _build_constants(thetas, phis):
    thetas = np.asarray(thetas, np.float64)
    phis = np.asarray(phis, np.float64)
    Ft = _fixed_layer_matrices(thetas, phis)
    Fhat = [_expand_group(_realify(M)) for M in Ft]

    SWAP = np.zeros((128, 128))
    for a in range(2):
        for g in range(2):
            for b in range(16):
                for r in range(2):
                    SWAP[a * 64 + g * 32 + b * 2 + r,
                         a * 64 + g * 32 + b * 2 + (1 - r)] = 1.0

    # c_sig: [34, 128]; p-tile col = g_new*16 + t*8 + j
    c_sig = np.zeros((34, 128), np.float32)
    for g in range(8):
        for t in range(2):
            for j in range(8):
                col = g * 16 + t * 8 + j
                for w in range(4):
                    c_sig[g * 4 + w, col] = 0.5 * _s_w(j, w)
                if t == 0:
                    c_sig[32, col] = np.float32(PI / 2)

    def bld_T(T):
        B0 = np.zeros((128, 128))
        for a in range(2):
            for g_old in range(2):
                g_new = 2 * T + g_old
                for b in range(16):
                    j, sg = _j_of_b(b), _sgn_of_b(b)
                    srow = a * 64 + g_old * 32 + b * 2
                    B0[srow + 0, g_new * 16 + 0 * 8 + j] = KAPPA
                    B0[srow + 1, g_new * 16 + 1 * 8 + j] = -KAPPA * sg
        return Fhat[0] @ B0

    def bc_T(T):
        M = np.zeros((128, 128))
        for a in range(2):
            for g_old in range(2):
                g_new = 2 * T + g_old
                for b in range(16):
                    j = _j_of_b(b)
                    srow = a * 64 + g_old * 32 + b * 2
                    M[srow + 0, g_new * 16 + 0 * 8 + j] = 1.0
                    M[srow + 1, g_new * 16 + 0 * 8 + j] = 1.0
        return M

    def bs_T(T):
        M = np.zeros((128, 128))
        for a in range(2):
            for g_old in range(2):
                g_new = 2 * T + g_old
                for b in range(16):
                    j, sg = _j_of_b(b), _sgn_of_b(b)
                    srow = a * 64 + g_old * 32 + b * 2
                    M[srow + 0, g_new * 16 + 1 * 8 + j] = -sg
                    M[srow + 1, g_new * 16 + 1 * 8 + j] = +sg
        return M

    # ev via squares: 2*sum(u*v) = 0.5*sum((u+v)^2 - (u-v)^2); P folds u+-v
    # into the last layer's matmuls so ACT Square reads psi3 straight from PSUM.
    P = np.zeros((128, 128))
    P[0:64, 0:64] = np.eye(64)
    P[0:64, 64:128] = np.eye(64)
    P[64:128, 0:64] = np.eye(64)
    P[64:128, 64:128] = -np.eye(64)

    c_ev = np.zeros((4, 128, 8), np.float16)
    for sl in range(4):
        for g in range(2):
            c_ev[sl, g * 32:(g + 1) * 32, 2 * sl + g] = 0.5
            c_ev[sl, 64 + g * 32:64 + (g + 1) * 32, 2 * sl + g] = -0.5

    return dict(
        c_sig=np.ascontiguousarray(c_sig),
        c_bld=np.ascontiguousarray(
            np.stack([bld_T(T).T for T in range(4)]).astype(np.float16)),
        c_bc=np.ascontiguousarray(
            np.stack([bc_T(T).T for T in range(4)]).astype(np.float16)),
        c_bs=np.ascontiguousarray(
            np.stack([bs_T(T).T for T in range(4)]).astype(np.float16)),
        c_f=np.ascontiguousarray(
            np.stack([Fhat[1].T, (Fhat[1] @ SWAP).T,
                      (P @ Fhat[2]).T, (P @ Fhat[2] @ SWAP).T]).astype(np.float16)),
        c_ev=np.ascontiguousarray(c_ev),
    )


def _angle_blocks(pix):
    """pix [P, 12] f32 -> A [P/4096 pairs, 3, 34, 512] f32.
    Patch linear order ((m*2+tt)*2+g)*512+n; pair P holds m in {2P, 2P+1};
    p-tile group g_new = (m%2)*4 + tt*2 + g; row g_new*4+w = angle of wire w,
    row 32 = 1.0 (pi/2 offset selector)."""
    n_pair = pix.shape[0] // 4096
    px = pix.reshape(n_pair, 2, 2, 2, 512, 12)   # (pair, m', tt, g, n, 12)
    A = np.zeros((n_pair, 3, 34, 512), np.float32)
    for l in range(3):
        for mp in range(2):
            for tt in range(2):
                for g in range(2):
                    g_new = 4 * mp + 2 * tt + g
                    for w in range(4):
                        A[:, l, g_new * 4 + w, :] = px[:, mp, tt, g, :, 4 * l + w]
        A[:, l, 32, :] = 1.0
    return A


# ----------------------------------------------------------------------------
# device program
# ----------------------------------------------------------------------------

def _build_nc(n_tiles=TILES_PER_CORE, repeat=1):
    """n_tiles = old 1024-patch tiles per core; must be divisible by 4."""
    import contextlib
    import concourse.mybir as mybir
    from concourse import bacc
    from concourse.tile import TileContext

    F32 = mybir.dt.float32
    F32R = mybir.dt.float32r
    F16 = mybir.dt.float16
    AF = mybir.ActivationFunctionType

    assert n_tiles % 4 == 0
    n_pair = n_tiles // 4

    nc = bacc.Bacc(None, target_bir_lowering=False, debug=False)
    ang_d = nc.declare_dram_parameter("ang", [n_pair, 3, 34, 512], F32R,
                                      isOutput=False)
    csig_d = nc.declare_dram_parameter("c_sig", [34, 128], F32R, isOutput=False)
    cbld_d = nc.declare_dram_parameter("c_bld", [4, 128, 128], F16, isOutput=False)
    cbc_d = nc.declare_dram_parameter("c_bc", [4, 128, 128], F16, isOutput=False)
    cbs_d = nc.declare_dram_parameter("c_bs", [4, 128, 128], F16, isOutput=False)
    cf_d = nc.declare_dram_parameter("c_f", [4, 128, 128], F16, isOutput=False)
    cev_d = nc.declare_dram_parameter("c_ev", [4, 128, 8], F16, isOutput=False)
    ev_d = nc.declare_dram_parameter("ev", [n_pair, 8, 512], F32, isOutput=True)

    BA = int(os.environ.get("BUFS_ANG", "6"))
    BP = int(os.environ.get("BUFS_P", "3"))
    BW = int(os.environ.get("BUFS_WRK", "3"))
    BC = int(os.environ.get("BUFS_CS", "4"))
    BS = int(os.environ.get("BUFS_PSIS", "3"))
    BM = int(os.environ.get("BUFS_MMT", "4"))
    # how many of each 4 consecutive bc/bs staging copies run on DVE (rest ACT)
    NDVE_CS = int(os.environ.get("NDVE_CS", "2"))
    # 1 = run the m2 multiply of every mh-layer on gpsimd instead of DVE
    NGP_MUL = int(os.environ.get("NGP_MUL", "0"))
    DO_WRAP = os.environ.get("NO_WRAP", "0") != "1"

    with TileContext(nc) as tc:
        with (
            tc.tile_pool(name="const", bufs=1) as cpool,
            tc.tile_pool(name="angp", bufs=BA) as angp,
            tc.tile_pool(name="wrk", bufs=BW) as wrk,
            tc.tile_pool(name="pp", bufs=BP) as pp,
            tc.tile_pool(name="csp", bufs=BC) as csp,
            tc.tile_pool(name="psis", bufs=BS) as psis,
            tc.tile_pool(name="mmt", bufs=BM) as mmt,
            tc.tile_pool(name="qp", bufs=2) as qp,
            tc.tile_pool(name="evs", bufs=2) as evs,
            tc.tile_pool(name="sigp", bufs=1, space="PSUM") as sigp,
            tc.tile_pool(name="bcbs", bufs=2, space="PSUM") as bcbs,
            tc.tile_pool(name="psip", bufs=2, space="PSUM") as psip,
            tc.tile_pool(name="evp", bufs=1, space="PSUM") as evp,
        ):
            c_sig = cpool.tile([34, 128], F32R)
            nc.sync.dma_start(out=c_sig[:], in_=csig_d[:])
            c_bld = []
            c_bc = []
            c_bs = []
            for T in range(4):
                tb = cpool.tile([128, 128], F16, tag=f"bld{T}")
                nc.sync.dma_start(out=tb[:], in_=cbld_d[T])
                c_bld.append(tb)
                t1 = cpool.tile([128, 128], F16, tag=f"bc{T}")
                nc.sync.dma_start(out=t1[:], in_=cbc_d[T])
                c_bc.append(t1)
                t2 = cpool.tile([128, 128], F16, tag=f"bs{T}")
                nc.sync.dma_start(out=t2[:], in_=cbs_d[T])
                c_bs.append(t2)
            c_f = []
            for k in range(4):
                tf = cpool.tile([128, 128], F16, tag=f"f{k}")
                nc.sync.dma_start(out=tf[:], in_=cf_d[k])
                c_f.append(tf)
            c_ev = []
            for sl in range(4):
                te = cpool.tile([128, 8], F16, tag=f"ev{sl}")
                nc.sync.dma_start(out=te[:], in_=cev_d[sl])
                c_ev.append(te)

            rep_ctx = (tc.For_i(0, repeat, 1) if repeat > 1
                       else contextlib.nullcontext())
            with rep_ctx:
                for Pp in range(n_pair):
                    a_ls = []
                    for l in range(3):
                        a_l = angp.tile([34, 512], F32R, tag=f"ang{l}")
                        nc.gpsimd.dma_start(out=a_l[:], in_=ang_d[Pp, l])
                        a_ls.append(a_l)

                    psi_s = [None, None]      # per mh, fp16 SBUF [128,1024]
                    q2 = [None, None]         # per mh, fp16 squares [128,1024]
                    cs_k = 0                  # bc/bs staging copy round-robin
                    for l in range(3):
                        sig = sigp.tile([128, 512], F32, tag="sig")
                        nc.tensor.matmul(sig[:], c_sig[:], a_ls[l][:],
                                         start=True, stop=True)
                        if DO_WRAP:
                            w = wrk.tile([128, 512], F32, tag="w")
                            nc.vector.add_range_wrap(
                                w[:], sig[:], shift=0.0, bound=PI, period=2 * PI)
                            p = pp.tile([128, 512], F16, tag="p")
                            nc.scalar.activation(p[:], w[:], AF.Sin)
                        else:
                            p = pp.tile([128, 512], F16, tag="p")
                            nc.scalar.activation(p[:], sig[:], AF.Sin)

                        for mh in range(2):
                            T0 = 2 * mh
                            psi_p = [psip.tile([128, 512], F32, tag="psi",
                                               name=f"psi{ti}")
                                     for ti in range(2)]
                            if l == 0:
                                for ti, T in enumerate((T0, T0 + 1)):
                                    nc.tensor.matmul(psi_p[ti][:], c_bld[T][:],
                                                     p[:], start=True, stop=True)
                            else:
                                bcP = bcbs.tile([128, 1024], F32, tag="bcbs")
                                nc.tensor.matmul(bcP[:, 0:512], c_bc[T0][:],
                                                 p[:], start=True, stop=True)
                                nc.tensor.matmul(bcP[:, 512:1024], c_bc[T0 + 1][:],
                                                 p[:], start=True, stop=True)
                                bsP = bcbs.tile([128, 1024], F32, tag="bcbs")
                                nc.tensor.matmul(bsP[:, 0:512], c_bs[T0][:],
                                                 p[:], start=True, stop=True)
                                nc.tensor.matmul(bsP[:, 512:1024], c_bs[T0 + 1][:],
                                                 p[:], start=True, stop=True)
                                bc_s = csp.tile([128, 1024], F16, tag="cs")
                                bs_s = csp.tile([128, 1024], F16, tag="cs")
                                for src, dst in ((bcP, bc_s), (bsP, bs_s)):
                                    if cs_k % 4 < NDVE_CS:
                                        nc.vector.tensor_copy(dst[:], src[:])
                                    else:
                                        nc.scalar.copy(out=dst[:], in_=src[:])
                                    cs_k += 1
                                m1 = mmt.tile([128, 1024], F16, tag="m")
                                nc.vector.tensor_mul(m1[:], bc_s[:], psi_s[mh][:])
                                m2 = mmt.tile([128, 1024], F16, tag="m")
                                if NGP_MUL:
                                    nc.gpsimd.tensor_mul(m2[:], bs_s[:],
                                                         psi_s[mh][:])
                                else:
                                    nc.vector.tensor_mul(m2[:], bs_s[:],
                                                         psi_s[mh][:])
                                base = 2 * (l - 1)
                                for ti in range(2):
                                    sl_c = slice(512 * ti, 512 * ti + 512)
                                    nc.tensor.matmul(psi_p[ti][:], c_f[base][:],
                                                     m1[:, sl_c], start=True,
                                                     stop=False)
                                    nc.tensor.matmul(psi_p[ti][:],
                                                     c_f[base + 1][:],
                                                     m2[:, sl_c], start=False,
                                                     stop=True)
                            if l < 2:
                                ps_t = psis.tile([128, 1024], F16, tag="psis")
                                for ti in range(2):
                                    nc.scalar.copy(
                                        out=ps_t[:, 512 * ti:512 * ti + 512],
                                        in_=psi_p[ti][:])
                                psi_s[mh] = ps_t
                            else:
                                q_t = qp.tile([128, 1024], F16, tag="q2")
                                for ti in range(2):
                                    nc.scalar.activation(
                                        q_t[:, 512 * ti:512 * ti + 512],
                                        psi_p[ti][:], AF.Square)
                                q2[mh] = q_t

                    evt = evp.tile([8, 512], F32, tag="ev")
                    for mh in range(2):
                        for ti in range(2):
                            sl = 2 * mh + ti
                            nc.tensor.matmul(evt[:],
                                             c_ev[sl][:],
                                             q2[mh][:, 512 * ti:512 * ti + 512],
                                             start=(sl == 0), stop=(sl == 3))
                    ev_s = evs.tile([8, 512], F32, tag="evs")
                    nc.scalar.copy(out=ev_s[:], in_=evt[:])
                    nc.sync.dma_start(out=ev_d[Pp], in_=ev_s[:])

    nc.finalize()
    return nc


def _get_nc(repeat=_REPEAT):
    key = ("nc", repeat)
    if key not in _CACHE:
        _CACHE[key] = _build_nc(repeat=repeat)
    return _CACHE[key]


# ----------------------------------------------------------------------------
# entry point
# ----------------------------------------------------------------------------

def kernel(x, thetas, phis):
    from concourse.bass_utils import run_bass_kernel_spmd

    x = np.asarray(x, np.float32)
    thetas = np.asarray(thetas, np.float32)
    phis = np.asarray(phis, np.float32)
    B, C, H, W = x.shape
    H2, W2 = H // 2, W // 2
    pix = (x.reshape(B, 3, H2, 2, W2, 2)
             .transpose(0, 2, 4, 1, 3, 5)
             .reshape(B * H2 * W2, 12))

    A = _angle_blocks(pix)                    # [64 pairs, 3, 34, 512]
    consts = _build_constants(thetas, phis)
    per_core = A.shape[0] // N_CORES
    in_maps = [{"ang": np.ascontiguousarray(A[c * per_core:(c + 1) * per_core]),
                **consts} for c in range(N_CORES)]

    nc = _get_nc()
    res = run_bass_kernel_spmd(nc, in_maps, list(range(N_CORES)))
    # ev_d [n_pair, 8, 512]: row 2*sl+g of pair P -> old tile 4*P+sl
    evs = [res.results[c]["ev"].reshape(-1, 4, 2, 512).reshape(-1)
           for c in range(N_CORES)]
    ev = np.concatenate(evs)
    return ev.reshape(B, 1, H2, W2).astype(np.float32)
